# revision 1
# baseline (speedup 1.0000x reference)
"""Trainium2 Bass kernel for nn_MelDecoder: DDSP-style mel decoder.

Pure data-parallel over (batch, time-half) -> 8 cores, no collectives.
Numerics replicate XLA-CPU fp32 behavior where the output is chaotic:
- phase cumsum via XLA's recursive blocked-16 scan association, bit-exact
  (segmented tensor_tensor_scan + the same broadcast-add reconstruction);
- oscillator sin arguments reduced in the cycles domain with the fl(2pi*C)
  rounding term (delta) folded into the fractional cycle count;
- harmonic Nyquist mask replicated exactly via precomputed fp32 thresholds.
The two FIR filters run as DFT matmuls (mag->windowed-IR spectrum is a single
precomputed linear map), followed by overlap-add with group-delay crop.
"""
import numpy as np
from contextlib import ExitStack

import concourse.bass as bass
import concourse.bacc as bacc
import concourse.tile as tile
import concourse.mybir as mybir
from concourse.bass import IndirectOffsetOnAxis
from concourse.bass_utils import run_bass_kernel_spmd

F32 = mybir.dt.float32
I32 = mybir.dt.int32
AF = mybir.ActivationFunctionType
OP = mybir.AluOpType
AX = mybir.AxisListType

SR = 24000
HOP = 240
NH = 80
T = 500
B = 4
N = 120000
HALF = 60000
FW = 256          # padded frame window per core (250 own + halo, padded)
FPC = 250         # output frames per core
FFT_H, NB_H, IR_H = 766, 384, 510
OUT_H = HOP + IR_H - 1     # 749
FFT_N, NB_N, IR_N = 510, 256, 158
OUT_N = HOP + IR_N - 1     # 397
RC = 8                     # oscillator r-chunk

TWO_PI_F = float(np.float32(2.0 * np.pi))
NEG_PI_F = float(np.float32(-np.pi))
H_F = np.float32(2.0 * np.pi)


def _f32_and(x, mask):
    return np.frombuffer((np.frombuffer(np.float32(x).tobytes(), dtype=np.uint32) & np.uint32(mask)).tobytes(), dtype=np.float32)[0]


HH_F = _f32_and(H_F, 0xFFFFF000)
HL_F = np.float32(np.float32(H_F) - HH_F)
EPSH_F = np.float32(np.float64(H_F) - 2.0 * np.pi)
INV2PI_F = np.float32(1.0 / (2.0 * np.pi))
LN10_F = float(np.float32(np.log(10.0)))


# ---------------------------------------------------------------- host constants
def _upsample_consts():
    pos = (np.arange(N, dtype=np.float32) / np.float32(HOP)).astype(np.float32)
    i0 = np.floor(pos).astype(np.int64)
    frac = (pos - i0.astype(np.float32)).astype(np.float32)
    w0 = (np.float32(1.0) - frac).astype(np.float32)
    return frac.reshape(T, HOP), w0.reshape(T, HOP)


def _mask_thresholds():
    thr = np.zeros(NH, dtype=np.float32)
    half_sr = np.float32(12000.0)
    for i in range(NH):
        k = np.float32(i + 1)
        cand = np.float32(np.float64(12000.0) / np.float64(k))
        while np.float32(cand * k) >= half_sr:
            cand = np.nextafter(cand, -np.inf, dtype=np.float32)
        while np.float32(cand * k) < half_sr:
            cand = np.nextafter(cand, np.inf, dtype=np.float32)
        thr[i] = cand
    return thr


def _build_filter_mats(M, ir_size, fft_size, out_len):
    nb = fft_size // 2 + 1
    t = np.arange(ir_size)[None, :]
    fidx = np.arange(M)[:, None]
    Cir = np.cos(2 * np.pi * fidx * t / ir_size) / ir_size
    Cir[1:M - 1] *= 2.0
    win = np.hanning(ir_size)
    roll = ir_size // 2
    P = np.zeros((ir_size, ir_size))
    for tt in range(ir_size):
        P[(tt + roll) % ir_size, tt] = 1.0
    tt2 = np.arange(ir_size)[:, None]
    ff2 = np.arange(nb)[None, :]
    CirPW = Cir @ P @ np.diag(win)
    A = np.concatenate([CirPW @ np.cos(-2 * np.pi * tt2 * ff2 / fft_size),
                        CirPW @ np.sin(-2 * np.pi * tt2 * ff2 / fft_size)], axis=1)
    tt3 = np.arange(HOP)[:, None]
    D = np.concatenate([np.cos(-2 * np.pi * tt3 * ff2 / fft_size),
                        np.sin(-2 * np.pi * tt3 * ff2 / fft_size)], axis=1)
    tt4 = np.arange(out_len)[None, :]
    ff4 = np.arange(nb)[:, None]
    I_re = np.cos(2 * np.pi * ff4 * tt4 / fft_size) / fft_size
    I_im = -np.sin(2 * np.pi * ff4 * tt4 / fft_size) / fft_size
    I_re[1:nb - 1] *= 2.0
    I_im[1:nb - 1] *= 2.0
    I = np.concatenate([I_re, I_im], axis=0)
    return A.astype(np.float32), D.astype(np.float32), I.astype(np.float32)


def host_constants():
    frac, w0 = _upsample_consts()
    kv = np.arange(1, NH + 1, dtype=np.float32)
    amp = (np.float32(0.4) * (np.float32(1.0) / kv).astype(np.float32)).astype(np.float32)
    A_h, D_h, I_h = _build_filter_mats(256, IR_H, FFT_H, OUT_H)
    A_n, D_n, I_n = _build_filter_mats(80, IR_N, FFT_N, OUT_N)
    return dict(FRAC_full=frac, W0_full=w0, KROW=kv, THRROW=_mask_thresholds(),
                AMPROW=amp, IOTA128=np.arange(128, dtype=np.float32),
                A_h=A_h, D_h=D_h, I_h=I_h, A_n=A_n, D_n=D_n, I_n=I_n)


# ---------------------------------------------------------------- kernel build
def build(debug=False):
    nc = bacc.Bacc("TRN2", target_bir_lowering=False, debug=False)

    def din(name, shape, dt=F32):
        return nc.dram_tensor(name, list(shape), dt, kind="ExternalInput")

    f0_xp = din("f0_xp", [T + 1])
    f0_win = din("f0_win", [FW + 1])
    mel_win = din("mel_win", [FW, 80])
    phon_win = din("phon_win", [FW])
    sid1 = din("sid1", [1])
    lid1 = din("lid1", [1])
    noise_win = din("noise_win", [FW, HOP])
    framemask = din("framemask", [FW])
    offcol = din("offcol", [FW], I32)
    ptab = din("ptab", [128, 128])
    sgtab = din("sgtab", [10, 16])
    lgtab = din("lgtab", [5, 8])
    W1 = din("W1", [233, 256])
    b1 = din("b1", [256])
    W2 = din("W2", [256, 336])
    b2 = din("b2", [336])
    FRACf = din("FRAC_full", [T, HOP])
    W0f = din("W0_full", [T, HOP])
    FRACw = din("FRAC_win", [FW, HOP])
    W0w = din("W0_win", [FW, HOP])
    KROW = din("KROW", [NH])
    THRROW = din("THRROW", [NH])
    AMPROW = din("AMPROW", [NH])
    IOTA = din("IOTA128", [128])
    A_h = din("A_h", [256, 2 * NB_H])
    D_h = din("D_h", [HOP, 2 * NB_H])
    I_h = din("I_h", [2 * NB_H, OUT_H])
    A_n = din("A_n", [80, 2 * NB_N])
    D_n = din("D_n", [HOP, 2 * NB_N])
    I_n = din("I_n", [2 * NB_N, OUT_N])
    qb = nc.dram_tensor("qb", [120 * 1024], F32)
    l0d = nc.dram_tensor("l0d", [7680], F32)
    bp1d = nc.dram_tensor("bp1d", [480], F32)
    bp2d = nc.dram_tensor("bp2d", [30], F32)
    vd = nc.dram_tensor("vd", [7680], F32)
    cfb = nc.dram_tensor("cfb", [512 * HOP], F32)
    out_d = nc.dram_tensor("out", [FPC, HOP], F32, kind="ExternalOutput")
    if debug:
        dbg_C = nc.dram_tensor("dbg_C", [120, 1024], F32, kind="ExternalOutput")
        dbg_harm = nc.dram_tensor("dbg_harm", [FW, HOP], F32, kind="ExternalOutput")
        dbg_mag = nc.dram_tensor("dbg_mag", [336, FW], F32, kind="ExternalOutput")

    with tile.TileContext(nc) as tc, ExitStack() as ctx:
        cp = ctx.enter_context(tc.tile_pool(name="consts", bufs=1))
        wp = ctx.enter_context(tc.tile_pool(name="work", bufs=1))
        w2p = ctx.enter_context(tc.tile_pool(name="work2", bufs=2))
        op = ctx.enter_context(tc.tile_pool(name="osc", bufs=2))
        pp = ctx.enter_context(tc.tile_pool(name="psum", bufs=4, space="PSUM"))
        py = ctx.enter_context(tc.tile_pool(name="psumy", bufs=1, space="PSUM"))

        # ---------------- consts
        def crow(name, src, n):
            t_ = cp.tile([1, n], F32, tag=name, name=name)
            nc.sync.dma_start(t_[:], src.ap().unsqueeze(0))
            return t_
        k80 = crow("c_k1", KROW, NH)
        thr80 = crow("c_t1", THRROW, NH)
        amp80 = crow("c_a1", AMPROW, NH)

        def pbc(name, row, p=128):
            t_ = cp.tile([p, row.shape[-1]], F32, tag=name, name=name)
            nc.gpsimd.partition_broadcast(t_[:], row[:] if hasattr(row, 'shape') else row)
            return t_
        KT = pbc("c_k", k80)
        THT = pbc("c_t", thr80)
        AMT = pbc("c_a", amp80)
        iota_row = crow("c_ir", IOTA, 128)
        iota_col = cp.tile([128, 1], F32, tag="c_ic")
        nc.sync.dma_start(iota_col[:], bass.AP(IOTA, 0, [[1, 128], [1, 1]]))
        iota_rows = pbc("c_irs", iota_row)
        ident = cp.tile([128, 128], F32, tag="c_id")
        nc.vector.tensor_scalar(ident[:], iota_rows[:], iota_col[:], None, OP.is_equal)
        pibias = cp.tile([128, 1], F32, tag="c_pi")
        nc.vector.memset(pibias[:], NEG_PI_F)
        magp = cp.tile([128, 1], F32, tag="c_magp")
        nc.vector.memset(magp[:], float(2.0 ** 23))
        magn = cp.tile([128, 1], F32, tag="c_magn")
        nc.vector.memset(magn[:], -float(2.0 ** 23))

        def col_chunks(name, src, total):
            outs = []
            base = 0
            i = 0
            while base < total:
                rows = min(128, total - base)
                t_ = cp.tile([128, 1], F32, tag=f"{name}{i}", name=f"{name}{i}")
                if rows < 128:
                    nc.vector.memset(t_[:], 0.0)
                nc.sync.dma_start(t_[:rows], bass.AP(src, base, [[1, rows], [1, 1]]))
                outs.append(t_)
                base += rows
                i += 1
            return outs
        fmcol = col_chunks("c_fm", framemask, FW)
        b1c = col_chunks("c_b1", b1, 256)
        b2c = col_chunks("c_b2", b2, 336)

        def mat_chunks(name, src, rows_total, cols):
            outs = []
            base = 0
            i = 0
            while base < rows_total:
                rows = min(128, rows_total - base)
                t_ = cp.tile([rows, cols], F32, tag=f"{name}{i}", name=f"{name}{i}")
                nc.sync.dma_start(t_[:], src.ap()[base:base + rows, :])
                outs.append(t_)
                base += rows
                i += 1
            return outs
        Ah_t = mat_chunks("c_Ah", A_h, 256, 2 * NB_H)
        Dh_t = mat_chunks("c_Dh", D_h, HOP, 2 * NB_H)
        Ih_t = mat_chunks("c_Ih", I_h, 2 * NB_H, OUT_H)
        An_t = mat_chunks("c_An", A_n, 80, 2 * NB_N)
        Dn_t = mat_chunks("c_Dn", D_n, HOP, 2 * NB_N)
        In_t = mat_chunks("c_In", I_n, 2 * NB_N, OUT_N)
        W1_t = mat_chunks("c_W1", W1, 233, 256)
        W2_t = mat_chunks("c_W2", W2, 256, 336)
        ptab_t = mat_chunks("c_pt", ptab, 128, 128)[0]
        sg_t = mat_chunks("c_sg", sgtab, 10, 16)[0]
        lg_t = mat_chunks("c_lg", lgtab, 5, 8)[0]

        # ---------------- helpers
        def clean_col(tag, src_dram, offset, rows):
            dst = w2p.tile([128, 1], F32, tag=tag, name=tag)
            if rows < 128:
                nc.vector.memset(dst[:], 0.0)
            nc.sync.dma_start(dst[:rows], bass.AP(src_dram, offset, [[1, rows], [1, 1]]))
            nc.vector.tensor_scalar(dst[:], dst[:], 1000.0, None, OP.min)
            nc.vector.tensor_scalar(dst[:], dst[:], 0.0, None, OP.max)
            m = w2p.tile([128, 1], F32, tag="ccm")
            nc.vector.tensor_scalar(m[:], dst[:], 80.0, None, OP.is_ge)
            nc.vector.tensor_tensor(dst[:], dst[:], m[:], OP.mult)
            return dst

        def pitch_up_chunk(src_dram, w0_dram, fr_dram, base, rows, out_tile):
            p0 = clean_col("p0", src_dram, base, rows)
            p1 = clean_col("p1", src_dram, base + 1, rows)
            w0t = w2p.tile([128, HOP], F32, tag="w0t")
            nc.sync.dma_start(w0t[:rows], w0_dram.ap()[base:base + rows, :])
            frt = w2p.tile([128, HOP], F32, tag="frt")
            nc.sync.dma_start(frt[:rows], fr_dram.ap()[base:base + rows, :])
            t0 = w2p.tile([128, HOP], F32, tag="t0")
            nc.vector.tensor_scalar(t0[:rows], w0t[:rows], p0[0:rows, :], None, OP.mult)
            t1 = w2p.tile([128, HOP], F32, tag="t1")
            nc.vector.tensor_scalar(t1[:rows], frt[:rows], p1[0:rows, :], None, OP.mult)
            nc.vector.tensor_tensor(out_tile, t0[:rows], t1[:rows], OP.add)

        # ---------------- S1: full pitch chain -> q -> qb (DRAM)
        for (base, rows) in ((0, 128), (128, 128), (256, 128), (384, 116)):
            pu = w2p.tile([128, HOP], F32, tag="pu")
            pitch_up_chunk(f0_xp, W0f, FRACf, base, rows, pu[:rows])
            qt = w2p.tile([128, HOP], F32, tag="qt")
            # exact fp32 division by SR: q0 = p*r, then Markstein residual correction
            R_SR = float(np.float32(1.0) / np.float32(SR))
            nc.vector.tensor_scalar(qt[:rows], pu[:rows], R_SR, None, OP.mult)
            q0h = w2p.tile([128, HOP], F32, tag="q0h")
            nc.vector.tensor_scalar(q0h[:rows].bitcast(mybir.dt.uint32), qt[:rows].bitcast(mybir.dt.uint32),
                                    0xFFFFF000, None, OP.bitwise_and)
            q0l = w2p.tile([128, HOP], F32, tag="q0l")
            nc.vector.tensor_tensor(q0l[:rows], qt[:rows], q0h[:rows], OP.subtract)
            nc.vector.tensor_scalar(q0h[:rows], q0h[:rows], float(-SR), None, OP.mult)
            nc.vector.tensor_scalar(q0l[:rows], q0l[:rows], float(-SR), None, OP.mult)
            rho = w2p.tile([128, HOP], F32, tag="rho")
            nc.vector.tensor_tensor(rho[:rows], pu[:rows], q0h[:rows], OP.add)
            nc.vector.tensor_tensor(rho[:rows], rho[:rows], q0l[:rows], OP.add)
            nc.vector.tensor_scalar(rho[:rows], rho[:rows], R_SR, None, OP.mult)
            nc.vector.tensor_tensor(qt[:rows], qt[:rows], rho[:rows], OP.add)
            nc.sync.dma_start(bass.AP(qb, base * HOP, [[HOP, rows], [1, HOP]]), qt[:rows])
        zt = wp.tile([120, 24], F32, tag="zt")
        nc.vector.memset(zt[:], 0.0)
        nc.sync.dma_start(bass.AP(qb, 120000, [[24, 120], [1, 24]]), zt[:])

        # ---------------- S2: XLA blocked-16 cumsum on [120, 1024]
        qt2 = wp.tile([120, 1024], F32, tag="csA")
        nc.sync.dma_start(qt2[:], bass.AP(qb, 0, [[1024, 120], [1, 1024]]))
        sm = wp.tile([120, 1024], F32, tag="csB")
        nc.vector.memset(sm[:], 1.0)
        nc.vector.memset(sm[:][:, 0:1024:16], 0.0)
        s0 = wp.tile([120, 1024], F32, tag="csC")
        nc.vector.tensor_tensor_scan(s0[:], sm[:], qt2[:], 0.0, OP.mult, OP.add)
        s0c = wp.tile([120, 64], F32, tag="cs_s0c")
        nc.vector.tensor_copy(s0c[:], s0[:][:, 15:1024:16])
        nc.sync.dma_start(bass.AP(l0d, 0, [[64, 120], [1, 64]]), s0c[:])
        # level 1: scan of L0-block sums (7680 = 60 x 128), XLA-nested
        l0r = wp.tile([60, 128], F32, tag="cs_l0r")
        nc.sync.dma_start(l0r[:], bass.AP(l0d, 0, [[128, 60], [1, 128]]))
        in1 = wp.tile([60, 128], F32, tag="cs_in1")
        nc.vector.tensor_tensor_scan(in1[:], sm[0:60, 0:128], l0r[:], 0.0, OP.mult, OP.add)
        # level 2: scan of L1-block sums (480 = 30 x 16)
        in1c = wp.tile([60, 8], F32, tag="cs_in1c")
        nc.vector.tensor_copy(in1c[:], in1[:][:, 15:128:16])
        l1r = wp.tile([30, 16], F32, tag="cs_l1r")
        nc.sync.dma_start(l1r[:], in1c[:])
        in2 = wp.tile([30, 16], F32, tag="cs_in2")
        nc.vector.tensor_tensor_scan(in2[:], sm[0:30, 0:16], l1r[:], 0.0, OP.mult, OP.add)
        # level 3: scan of L2-block sums (30)
        l2r = wp.tile([1, 30], F32, tag="cs_l2r")
        nc.sync.dma_start(l2r[:], in2[:][:, 15:16])
        in3 = wp.tile([1, 30], F32, tag="cs_in3")
        nc.vector.tensor_tensor_scan(in3[:], sm[0:1, 0:30], l2r[:], 0.0, OP.mult, OP.add)
        # bpref2 (inclusive scanned L2-sums, L3/L4-nested): x4p + in3
        x4p = wp.tile([1, 30], F32, tag="cs_x4")
        nc.vector.memset(x4p[:], 0.0)
        nc.vector.tensor_copy(x4p[:][:, 16:30], in3[:][:, 15:16].broadcast_to((1, 14)))
        bp2 = wp.tile([1, 30], F32, tag="cs_bp2")
        nc.vector.tensor_tensor(bp2[:], x4p[:], in3[:], OP.add)
        nc.sync.dma_start(bass.AP(bp2d, 0, [[30, 1], [1, 30]]), bp2[:])
        # bpref1 [30,16] = fl(bp2shift_col + in2)
        bp2s = wp.tile([30, 1], F32, tag="cs_bp2s")
        nc.vector.memset(bp2s[:], 0.0)
        nc.sync.dma_start(bp2s[1:30, :], bass.AP(bp2d, 0, [[1, 29], [1, 1]]))
        bp1 = wp.tile([30, 16], F32, tag="cs_bp1")
        nc.vector.tensor_scalar(bp1[:], in2[:], bp2s[:], None, OP.add)
        nc.sync.dma_start(bass.AP(bp1d, 0, [[16, 30], [1, 16]]), bp1[:])
        # bpref0 [60,128] = fl(bp1shift_grp + in1)
        bp1s = wp.tile([60, 8], F32, tag="cs_bp1s")
        nc.vector.memset(bp1s[:], 0.0)
        nc.sync.dma_start(bp1s[:, 1:8], bass.AP(bp1d, 0, [[8, 60], [1, 7]]))
        nc.sync.dma_start(bp1s[1:60, 0:1], bass.AP(bp1d, 7, [[8, 59], [1, 1]]))
        bp0 = wp.tile([60, 128], F32, tag="cs_bp0")
        nc.vector.tensor_tensor(bp0[:].rearrange("p (g j) -> p g j", j=16),
                                in1[:].rearrange("p (g j) -> p g j", j=16),
                                bp1s[:].unsqueeze(2).broadcast_to((60, 8, 16)), OP.add)
        # V[b0] = bp0[b0-1] (global shift by one block)
        vt = wp.tile([60, 128], F32, tag="cs_vt")
        nc.vector.memset(vt[:][:, 0:1], 0.0)
        nc.vector.tensor_copy(vt[:][:, 1:128], bp0[:][:, 0:127])
        nc.sync.dma_start(vt[1:60, 0:1], bp0[0:59, 127:128])
        nc.sync.dma_start(bass.AP(vd, 0, [[128, 60], [1, 128]]), vt[:])
        vcol = wp.tile([120, 64], F32, tag="cs_vcol")
        nc.sync.dma_start(vcol[:], bass.AP(vd, 0, [[64, 120], [1, 64]]))
        Ct = wp.tile([120, 1024], F32, tag="csD")
        nc.vector.tensor_tensor(Ct[:].rearrange("p (g j) -> p g j", j=16),
                                s0[:].rearrange("p (g j) -> p g j", j=16),
                                vcol[:].unsqueeze(2).broadcast_to((120, 64, 16)), OP.add)
        if debug:
            nc.sync.dma_start(dbg_C.ap(), Ct[:])

        # ---------------- S3: dd + Cf2 -> cfb
        phi = wp.tile([120, 1024], F32, tag="csA")
        nc.vector.tensor_scalar(phi[:], Ct[:], float(H_F), None, OP.mult)
        ch = wp.tile([120, 1024], F32, tag="csB")
        nc.vector.tensor_scalar(ch[:].bitcast(mybir.dt.uint32), Ct[:].bitcast(mybir.dt.uint32),
                                0xFFFFF000, None, OP.bitwise_and)
        cl = wp.tile([120, 1024], F32, tag="csC")
        nc.vector.tensor_tensor(cl[:], Ct[:], ch[:], OP.subtract)
        e = wp.tile([120, 1024], F32, tag="csE")
        nc.vector.tensor_scalar(e[:], ch[:], float(HH_F), None, OP.mult)
        nc.vector.tensor_tensor(e[:], e[:], phi[:], OP.subtract)
        tmp = wp.tile([120, 1024], F32, tag="csF")
        nc.vector.tensor_scalar(tmp[:], cl[:], float(HH_F), None, OP.mult)
        nc.vector.tensor_tensor(e[:], e[:], tmp[:], OP.add)
        nc.vector.tensor_scalar(tmp[:], ch[:], float(HL_F), None, OP.mult)
        nc.vector.tensor_tensor(e[:], e[:], tmp[:], OP.add)
        nc.vector.tensor_scalar(tmp[:], cl[:], float(HL_F), None, OP.mult)
        nc.vector.tensor_tensor(e[:], e[:], tmp[:], OP.add)
        nc.vector.tensor_scalar(tmp[:], Ct[:], float(EPSH_F), None, OP.mult)
        nc.vector.tensor_tensor(tmp[:], tmp[:], e[:], OP.subtract)
        nc.vector.tensor_scalar(tmp[:], tmp[:], float(INV2PI_F), None, OP.mult)
        cfr = wp.tile([120, 1024], F32, tag="csD2")
        fl_ = wp.tile([120, 1024], F32, tag="csFL")
        nc.vector.tensor_scalar(fl_[:], Ct[:], float(2.0 ** 23), None, OP.add)
        nc.vector.tensor_scalar(fl_[:], fl_[:], float(2.0 ** 23), None, OP.subtract)
        gg = wp.tile([120, 1024], F32, tag="csGG")
        nc.vector.tensor_tensor(gg[:], fl_[:], Ct[:], OP.is_gt)
        nc.vector.tensor_tensor(fl_[:], fl_[:], gg[:], OP.subtract)
        nc.vector.tensor_tensor(cfr[:], Ct[:], fl_[:], OP.subtract)
        nc.vector.tensor_tensor(cfr[:], cfr[:], tmp[:], OP.add)
        nc.sync.dma_start(bass.AP(cfb, 0, [[1024, 117], [1, 1024]]), cfr[0:117])
        nc.sync.dma_start(bass.AP(cfb, 117 * 1024, [[1024, 1], [1, 192]]), cfr[117:118, 0:192])
        nc.sync.dma_start(bass.AP(cfb, 120000, [[24, 120], [1, 24]]), zt[:])

        # ---------------- S4: window pitch_up + Cf2 gather + oscillator
        M1 = [wp.tile([128, HOP], F32, tag=f"m1_{fc}", name=f"m1_{fc}") for fc in range(2)]
        for fc in range(2):
            base = fc * 128
            puw = wp.tile([128, HOP], F32, tag=f"puw{fc}")
            pitch_up_chunk(f0_win, W0w, FRACw, base, 128, puw[:])
            oc_ = wp.tile([128, 1], I32, tag=f"oc{fc}")
            nc.sync.dma_start(oc_[:], bass.AP(offcol, base, [[1, 128], [1, 1]]))
            cfw = wp.tile([128, HOP], F32, tag=f"cfw{fc}")
            nc.gpsimd.indirect_dma_start(
                cfw[:], None, bass.AP(cfb, 0, [[HOP, 512], [1, HOP]]),
                IndirectOffsetOnAxis(ap=oc_[:], axis=0))
            for rc in range(HOP // RC):
                rsl = slice(rc * RC, (rc + 1) * RC)
                cf_b = cfw[:][:, rsl].unsqueeze(2).broadcast_to((128, RC, NH))
                pu_b = puw[:][:, rsl].unsqueeze(2).broadcast_to((128, RC, NH))
                kt_b = KT[:].unsqueeze(1).broadcast_to((128, RC, NH))
                th_b = THT[:].unsqueeze(1).broadcast_to((128, RC, NH))
                am_b = AMT[:].unsqueeze(1).broadcast_to((128, RC, NH))
                wt = op.tile([128, RC * NH], F32, tag="o_A")
                nc.vector.tensor_tensor(wt[:].rearrange("p (r k) -> p r k", k=NH), cf_b, kt_b, OP.mult)
                zr = op.tile([128, RC * NH], F32, tag="o_B")
                nc.gpsimd.tensor_scalar(zr[:], wt[:], float(2.0 ** 23), None, OP.add)
                nc.gpsimd.tensor_scalar(zr[:], zr[:], float(2.0 ** 23), None, OP.subtract)
                fr = op.tile([128, RC * NH], F32, tag="o_D")
                nc.vector.tensor_tensor(fr[:], wt[:], zr[:], OP.subtract)
                sn = op.tile([128, RC * NH], F32, tag="o_A")
                nc.scalar.activation(sn[:], fr[:], AF.Sin, scale=TWO_PI_F)
                mt = op.tile([128, RC * NH], F32, tag="o_C")
                nc.vector.tensor_tensor(mt[:].rearrange("p (r k) -> p r k", k=NH), pu_b, th_b, OP.is_lt)
                sel = op.tile([128, RC * NH], F32, tag="o_C")
                nc.vector.tensor_tensor(sel[:].rearrange("p (r k) -> p r k", k=NH),
                                        mt[:].rearrange("p (r k) -> p r k", k=NH), am_b, OP.mult)
                pt = op.tile([128, RC * NH], F32, tag="o_B")
                nc.vector.tensor_tensor(pt[:], sn[:], sel[:], OP.mult)
                nc.vector.tensor_reduce(M1[fc][:][:, rsl], pt[:].rearrange("p (r k) -> p r k", k=NH),
                                        AX.X, OP.add)
        if debug:
            nc.sync.dma_start(dbg_harm.ap()[0:128, :], M1[0][:])
            nc.sync.dma_start(dbg_harm.ap()[128:256, :], M1[1][:])

        # ---------------- S5: framesT via PE transpose (harm + noise)
        def transpose_fw(src_tiles, name):
            d0 = wp.tile([128, FW], F32, tag=f"{name}0")
            d1 = wp.tile([112, FW], F32, tag=f"{name}1")
            for fc in range(2):
                ps = pp.tile([128, 128], F32, tag="ps")
                nc.tensor.transpose(ps[:], src_tiles[fc][:][:, 0:128], ident[:])
                nc.scalar.copy(d0[:][:, fc * 128:(fc + 1) * 128], ps[:])
                ps2 = pp.tile([112, 128], F32, tag="ps")
                nc.tensor.transpose(ps2[:], src_tiles[fc][:][:, 128:240], ident[:])
                nc.scalar.copy(d1[:][:, fc * 128:(fc + 1) * 128], ps2[:])
            return d0, d1

        HFT0, HFT1 = transpose_fw(M1, "hft")
        NZM = [wp.tile([128, HOP], F32, tag=f"nzm{fc}", name=f"nzm{fc}") for fc in range(2)]
        for fc in range(2):
            nz = w2p.tile([128, HOP], F32, tag="nzin")
            nc.sync.dma_start(nz[:], noise_win.ap()[fc * 128:(fc + 1) * 128, :])
            nc.vector.tensor_scalar(NZM[fc][:], nz[:], 2.0, None, OP.mult)
            nc.vector.tensor_scalar(NZM[fc][:], NZM[fc][:], 1.0, None, OP.subtract)
        NFT0, NFT1 = transpose_fw(NZM, "nft")

        # ---------------- S6: MLP -> magT
        melT = wp.tile([80, FW], F32, tag="melT")
        for fc in range(2):
            melc = w2p.tile([128, 80], F32, tag="melc")
            nc.sync.dma_start(melc[:], mel_win.ap()[fc * 128:(fc + 1) * 128, :])
            ps = pp.tile([80, 128], F32, tag="ps")
            nc.tensor.transpose(ps[:], melc[:], ident[:])
            nc.scalar.copy(melT[:][:, fc * 128:(fc + 1) * 128], ps[:])
        f0row = wp.tile([1, FW], F32, tag="f0row")
        nc.sync.dma_start(f0row[:], bass.AP(f0_win, 0, [[FW, 1], [1, FW]]))
        phrow = wp.tile([1, FW], F32, tag="phrow")
        nc.sync.dma_start(phrow[:], bass.AP(phon_win, 0, [[FW, 1], [1, FW]]))
        phrows = wp.tile([128, FW], F32, tag="phrows")
        nc.gpsimd.partition_broadcast(phrows[:], phrow[:])
        onehot = wp.tile([128, FW], F32, tag="onehot")
        nc.vector.tensor_scalar(onehot[:], phrows[:], iota_col[:], None, OP.is_equal)
        phps = pp.tile([128, FW], F32, tag="ps")
        nc.tensor.matmul(phps[:], ptab_t[:], onehot[:], start=True, stop=True)
        phT = wp.tile([128, FW], F32, tag="phT")
        nc.scalar.copy(phT[:], phps[:])

        def emb_bcast(tab_tile, idx_dram, nrows, dim, name):
            idxb = wp.tile([nrows, 1], F32, tag=f"{name}i")
            nc.sync.dma_start(idxb[:], bass.AP(idx_dram, 0, [[0, nrows], [1, 1]]))
            oh = wp.tile([nrows, 1], F32, tag=f"{name}o")
            nc.vector.tensor_scalar(oh[:], iota_col[0:nrows, :], idxb[:], None, OP.is_equal)
            vps = pp.tile([dim, 1], F32, tag="ps")
            nc.tensor.matmul(vps[:], tab_tile[:], oh[:], start=True, stop=True)
            vcol = wp.tile([dim, 1], F32, tag=f"{name}c")
            nc.scalar.copy(vcol[:], vps[:])
            vT = wp.tile([dim, FW], F32, tag=f"{name}T")
            nc.vector.tensor_copy(vT[:], vcol[:].broadcast_to((dim, FW)))
            return vT
        sgT = emb_bcast(sg_t, sid1, 10, 16, "sg")
        lgT = emb_bcast(lg_t, lid1, 5, 8, "lg")

        # assemble featsT: chunk0 = [mel(80) | f0(1) | ph 0:47], chunk1 = [ph 47:128 | sg | lg]
        ft0 = wp.tile([128, FW], F32, tag="ft0")
        ft1 = wp.tile([105, FW], F32, tag="ft1")
        nc.sync.dma_start(ft0[0:80, :], melT[:])
        nc.sync.dma_start(ft0[80:81, :], f0row[:])
        nc.sync.dma_start(ft0[81:128, :], phT[0:47, :])
        nc.sync.dma_start(ft1[0:81, :], phT[47:128, :])
        nc.sync.dma_start(ft1[81:97, :], sgT[:])
        nc.sync.dma_start(ft1[97:105, :], lgT[:])
        HT = [wp.tile([128, FW], F32, tag=f"HT{mc}", name=f"HT{mc}") for mc in range(2)]
        for mc in range(2):
            msl = slice(mc * 128, (mc + 1) * 128)
            hps = pp.tile([128, FW], F32, tag="ps")
            nc.tensor.matmul(hps[:], W1_t[0][:, msl], ft0[:], start=True, stop=False)
            nc.tensor.matmul(hps[:], W1_t[1][0:105, msl], ft1[:], start=False, stop=True)
            nc.scalar.activation(HT[mc][:], hps[:], AF.Relu, bias=b1c[mc][:], scale=1.0)
        magT = [wp.tile([128, FW], F32, tag=f"magT{mc}", name=f"magT{mc}") for mc in range(3)]
        for mc, rows in enumerate((128, 128, 80)):
            msl = slice(mc * 128, mc * 128 + rows)
            cps = pp.tile([rows, FW], F32, tag="ps")
            nc.tensor.matmul(cps[:], W2_t[0][:, msl], HT[0][:], start=True, stop=False)
            nc.tensor.matmul(cps[:], W2_t[1][:, msl], HT[1][:], start=False, stop=True)
            mg = magT[mc][0:rows, :]
            nc.scalar.activation(mg, cps[:], AF.Sigmoid, bias=b2c[mc][0:rows, :], scale=1.0)
            nc.scalar.activation(mg, mg, AF.Ln)
            nc.scalar.activation(mg, mg, AF.Exp, scale=LN10_F)
            nc.scalar.activation(mg, mg, AF.Copy, bias=1e-7, scale=2.0)
        if debug:
            for mc, rows in enumerate((128, 128, 80)):
                nc.sync.dma_start(dbg_mag.ap()[mc * 128:mc * 128 + rows, :], magT[mc][0:rows, :])

        # ---------------- S7: filters
        def spectrum(lhs, lhs_rows, rhs, nchunks, name):
            outs = []
            for mc in range(nchunks):
                msl = slice(mc * 128, (mc + 1) * 128)
                ps = pp.tile([128, FW], F32, tag="ps")
                for k in range(len(lhs)):
                    nc.tensor.matmul(ps[:], lhs[k][0:lhs_rows[k], msl], rhs[k],
                                     start=(k == 0), stop=(k == len(lhs) - 1))
                o = wp.tile([128, FW], F32, tag=f"{name}{mc}", name=f"{name}{mc}")
                nc.scalar.copy(o[:], ps[:])
                outs.append(o)
            return outs

        SIR_h = spectrum(Ah_t, [128, 128], [magT[0][:], magT[1][:]], 6, "sirh")
        SFR_h = spectrum(Dh_t, [128, 112], [HFT0[:], HFT1[:]], 6, "sfrh")
        SIR_n = spectrum(An_t, [80], [magT[2][0:80, :]], 4, "sirn")
        SFR_n = spectrum(Dn_t, [128, 112], [NFT0[:], NFT1[:]], 4, "sfrn")

        def cmul(a, b, nre, name):
            outs = []
            for c in range(nre * 2):
                outs.append(wp.tile([128, FW], F32, tag=f"{name}{c}", name=f"{name}{c}"))
            for c in range(nre):
                t1_ = w2p.tile([128, FW], F32, tag=f"{name}t1")
                t2_ = w2p.tile([128, FW], F32, tag=f"{name}t2")
                nc.vector.tensor_tensor(t1_[:], a[c][:], b[c][:], OP.mult)
                nc.vector.tensor_tensor(t2_[:], a[c + nre][:], b[c + nre][:], OP.mult)
                nc.vector.tensor_tensor(outs[c][:], t1_[:], t2_[:], OP.subtract)
                t3_ = w2p.tile([128, FW], F32, tag=f"{name}t1")
                t4_ = w2p.tile([128, FW], F32, tag=f"{name}t2")
                nc.vector.tensor_tensor(t3_[:], a[c][:], b[c + nre][:], OP.mult)
                nc.vector.tensor_tensor(t4_[:], a[c + nre][:], b[c][:], OP.mult)
                nc.vector.tensor_tensor(outs[c + nre][:], t3_[:], t4_[:], OP.add)
            return outs

        SY_h = cmul(SIR_h, SFR_h, 3, "cmh")
        SY_n = cmul(SIR_n, SFR_n, 2, "cmn")

        def irfft_y(SY, I_tiles, out_len, name):
            Ysb = []
            for fc in range(2):
                fsl = slice(fc * 128, (fc + 1) * 128)
                yp = py.tile([128, out_len], F32, tag=f"yp_{name}")
                for ns in range(0, out_len, 512):
                    ne = min(out_len, ns + 512)
                    for k in range(len(SY)):
                        nc.tensor.matmul(yp[:][:, ns:ne], SY[k][:][:, fsl], I_tiles[k][:][:, ns:ne],
                                         start=(k == 0), stop=(k == len(SY) - 1))
                o = wp.tile([128, out_len], F32, tag=f"{name}sb{fc}", name=f"{name}sb{fc}")
                nc.vector.tensor_scalar(o[:], yp[:], fmcol[fc][:], None, OP.mult)
                Ysb.append(o)
            return Ysb

        Yh = irfft_y(SY_h, Ih_t, OUT_H, "yh")
        Yn = irfft_y(SY_n, In_t, OUT_N, "yn")

        # ---------------- S8: OLA + output
        for oc_i, orows in ((0, 128), (1, 122)):
            F0 = oc_i * 128
            acc = wp.tile([128, HOP], F32, tag=f"acc{oc_i}", name=f"acc{oc_i}")
            nc.vector.memset(acc[:], 0.0)

            def add_contrib(Y, j, d, out_len):
                pos0 = HOP * j + d
                r0, r1 = max(0, -pos0), min(HOP, out_len - pos0)
                if r0 >= r1:
                    return
                g0_ = F0 + 2 - j
                sh = w2p.tile([128, HOP], F32, tag="olash", name="olash")
                nc.vector.memset(sh[:], 0.0)
                for part in range(2):
                    lo = max(g0_, part * 128) - g0_
                    hi = min(g0_ + orows, (part + 1) * 128) - g0_
                    if lo >= hi:
                        continue
                    nc.sync.dma_start(
                        sh[lo:hi, r0:r1],
                        Y[part][:][g0_ + lo - part * 128: g0_ + hi - part * 128,
                                   pos0 + r0: pos0 + r1])
                nc.vector.tensor_tensor(acc[:], acc[:], sh[:], OP.add)

            for j in (-2, -1, 0, 1, 2):
                add_contrib(Yh, j, IR_H // 2, OUT_H)
            for j in (-1, 0, 1):
                add_contrib(Yn, j, IR_N // 2, OUT_N)
            nc.sync.dma_start(out_d.ap()[F0:F0 + orows, :], acc[0:orows, :])

    nc.compile()
    return nc


# ---------------------------------------------------------------- host driver
_CACHE = {}


def _get_nc(debug=False):
    key = ("nc", debug)
    if key not in _CACHE:
        _CACHE[key] = build(debug=debug)
    return _CACHE[key]


def make_in_maps(inputs, consts=None):
    consts = consts or host_constants()
    f32 = np.float32
    mel = np.asarray(inputs["mel"]).astype(f32)
    f0 = np.asarray(inputs["f0"]).astype(f32)
    phon = np.asarray(inputs["phoneme_seq"]).astype(f32)
    noise = np.asarray(inputs["noise"]).astype(f32)
    ptab = np.zeros((128, 128), f32)
    ptab[:101] = np.asarray(inputs["phoneme_table"]).astype(f32)
    in_maps = []
    for c in range(8):
        b, h = c // 2, c % 2
        g0 = h * FPC - 2
        gidx = np.arange(FW) + g0
        valid = (gidx >= 0) & (gidx < T)
        gcl = np.clip(gidx, 0, T - 1)
        xp = np.concatenate([f0[b], f0[b, -1:]])
        f0w = np.zeros(FW + 1, f32)
        gi2 = np.arange(FW + 1) + g0
        v2 = (gi2 >= 0) & (gi2 < T + 1)
        f0w[v2] = xp[np.clip(gi2, 0, T)][v2]
        melw = np.zeros((FW, 80), f32); melw[valid] = mel[b][gcl[valid]]
        phw = np.zeros(FW, f32); phw[valid] = phon[b][gcl[valid]]
        nzw = np.zeros((FW, HOP), f32)
        nzw[valid] = noise[b].reshape(T, HOP)[gcl[valid]]
        fm = valid.astype(f32)
        m = dict(
            f0_xp=xp.astype(f32), f0_win=f0w, mel_win=melw, phon_win=phw,
            sid1=np.asarray(inputs["singer_id"]).astype(f32)[b:b + 1].copy(),
            lid1=np.asarray(inputs["language_id"]).astype(f32)[b:b + 1].copy(),
            noise_win=nzw, framemask=fm,
            offcol=gcl.astype(np.int32),
            ptab=ptab,
            sgtab=np.asarray(inputs["singer_table"]).astype(f32),
            lgtab=np.asarray(inputs["language_table"]).astype(f32),
            W1=np.asarray(inputs["W1"]).astype(f32), b1=np.asarray(inputs["b1"]).astype(f32),
            W2=np.asarray(inputs["W2"]).astype(f32), b2=np.asarray(inputs["b2"]).astype(f32),
            FRAC_full=consts["FRAC_full"], W0_full=consts["W0_full"],
            FRAC_win=(consts["FRAC_full"][gcl] * fm[:, None]).astype(f32),
            W0_win=(consts["W0_full"][gcl] * fm[:, None]).astype(f32),
            KROW=consts["KROW"], THRROW=consts["THRROW"], AMPROW=consts["AMPROW"],
            IOTA128=consts["IOTA128"],
            A_h=consts["A_h"], D_h=consts["D_h"], I_h=consts["I_h"],
            A_n=consts["A_n"], D_n=consts["D_n"], I_n=consts["I_n"],
        )
        in_maps.append(m)
    return in_maps


def kernel(**inputs):
    nc = _get_nc(debug=False)
    in_maps = make_in_maps(inputs)
    res = run_bass_kernel_spmd(nc, in_maps, list(range(8)))
    out = np.zeros((B, N), np.float32)
    for c in range(8):
        b, h = c // 2, c % 2
        out[b, h * HALF:(h + 1) * HALF] = res.results[c]["out"].reshape(HALF)
    return out



# revision 36
# speedup vs baseline: 1.0380x; 1.0380x over previous
"""Trainium2 Bass kernel for nn_MelDecoder: DDSP-style mel decoder.

Pure data-parallel over (batch, time-half) -> 8 cores, no collectives.
Numerics replicate XLA-CPU fp32 behavior where the output is chaotic:
- phase cumsum via XLA's recursive blocked-16 scan association, bit-exact
  (segmented tensor_tensor_scan + the same broadcast-add reconstruction);
- oscillator sin arguments reduced in the cycles domain with the fl(2pi*C)
  rounding term (delta) folded into the fractional cycle count;
- harmonic Nyquist mask replicated exactly via precomputed fp32 thresholds.
The two FIR filters run as DFT matmuls (mag->windowed-IR spectrum is a single
precomputed linear map), followed by overlap-add with group-delay crop.
"""
import numpy as np
from contextlib import ExitStack

import concourse.bass as bass
import concourse.bacc as bacc
import concourse.tile as tile
import concourse.mybir as mybir
from concourse.bass import IndirectOffsetOnAxis
from concourse.bass_utils import run_bass_kernel_spmd

F32 = mybir.dt.float32
I32 = mybir.dt.int32
AF = mybir.ActivationFunctionType
OP = mybir.AluOpType
AX = mybir.AxisListType

SR = 24000
HOP = 240
NH = 80
T = 500
B = 4
N = 120000
HALF = 60000
FW = 256          # padded frame window per core (250 own + halo, padded)
FPC = 250         # output frames per core
FFT_H, NB_H, IR_H = 766, 384, 510
OUT_H = HOP + IR_H - 1     # 749
FFT_N, NB_N, IR_N = 510, 256, 158
OUT_N = HOP + IR_N - 1     # 397
RC = 8                     # oscillator r-chunk

TWO_PI_F = float(np.float32(2.0 * np.pi))
NEG_PI_F = float(np.float32(-np.pi))
H_F = np.float32(2.0 * np.pi)


def _f32_and(x, mask):
    return np.frombuffer((np.frombuffer(np.float32(x).tobytes(), dtype=np.uint32) & np.uint32(mask)).tobytes(), dtype=np.float32)[0]


HH_F = _f32_and(H_F, 0xFFFFF000)
HL_F = np.float32(np.float32(H_F) - HH_F)
EPSH_F = np.float32(np.float64(H_F) - 2.0 * np.pi)
INV2PI_F = np.float32(1.0 / (2.0 * np.pi))
LN10_F = float(np.float32(np.log(10.0)))


# ---------------------------------------------------------------- host constants
def _upsample_consts():
    pos = (np.arange(N, dtype=np.float32) / np.float32(HOP)).astype(np.float32)
    i0 = np.floor(pos).astype(np.int64)
    frac = (pos - i0.astype(np.float32)).astype(np.float32)
    w0 = (np.float32(1.0) - frac).astype(np.float32)
    return frac.reshape(T, HOP), w0.reshape(T, HOP)


def _mask_thresholds():
    thr = np.zeros(NH, dtype=np.float32)
    half_sr = np.float32(12000.0)
    for i in range(NH):
        k = np.float32(i + 1)
        cand = np.float32(np.float64(12000.0) / np.float64(k))
        while np.float32(cand * k) >= half_sr:
            cand = np.nextafter(cand, -np.inf, dtype=np.float32)
        while np.float32(cand * k) < half_sr:
            cand = np.nextafter(cand, np.inf, dtype=np.float32)
        thr[i] = cand
    return thr


def _build_filter_mats(M, ir_size, fft_size, out_len):
    nb = fft_size // 2 + 1
    t = np.arange(ir_size)[None, :]
    fidx = np.arange(M)[:, None]
    Cir = np.cos(2 * np.pi * fidx * t / ir_size) / ir_size
    Cir[1:M - 1] *= 2.0
    win = np.hanning(ir_size)
    roll = ir_size // 2
    P = np.zeros((ir_size, ir_size))
    for tt in range(ir_size):
        P[(tt + roll) % ir_size, tt] = 1.0
    tt2 = np.arange(ir_size)[:, None]
    ff2 = np.arange(nb)[None, :]
    CirPW = Cir @ P @ np.diag(win)
    A = np.concatenate([CirPW @ np.cos(-2 * np.pi * tt2 * ff2 / fft_size),
                        CirPW @ np.sin(-2 * np.pi * tt2 * ff2 / fft_size)], axis=1)
    tt3 = np.arange(HOP)[:, None]
    D = np.concatenate([np.cos(-2 * np.pi * tt3 * ff2 / fft_size),
                        np.sin(-2 * np.pi * tt3 * ff2 / fft_size)], axis=1)
    tt4 = np.arange(out_len)[None, :]
    ff4 = np.arange(nb)[:, None]
    I_re = np.cos(2 * np.pi * ff4 * tt4 / fft_size) / fft_size
    I_im = -np.sin(2 * np.pi * ff4 * tt4 / fft_size) / fft_size
    I_re[1:nb - 1] *= 2.0
    I_im[1:nb - 1] *= 2.0
    I = np.concatenate([I_re, I_im], axis=0)
    return A.astype(np.float32), D.astype(np.float32), I.astype(np.float32)


def _osc_pack():
    """(block,k)-pair packing tables for the PE-centric oscillator.

    640 pairs = 8 blocks x 80 harmonics -> 5 chunks of 128 partitions.
    LK  [5][8,128]  : k value at (rhs-row=block, partition)     (exact in f16)
    LW  [5][16,128] : w16=f16(1/thr_k) at hi(0:8)+lo(8:16) rows (f16)
    T2  [5][128]    : exact f32 threshold in the w16-scaled domain
    LA  [5][128,8]  : f16(0.4/k) selector for the amp-weighted reduce
    """
    thr = _mask_thresholds()
    f16, f32 = np.float16, np.float32
    LK = np.zeros((5, 8, 128), f32)
    LW = np.zeros((5, 16, 128), f32)
    T2 = np.zeros((5, 128), f32)
    LA = np.zeros((5, 128, 8), f32)
    for c in range(5):
        for p in range(128):
            q = 128 * c + p
            b, k = q // 80, q % 80 + 1
            th = f32(thr[k - 1])
            w16 = f16(1.0 / np.float64(th))
            LK[c, b, p] = k
            LW[c, b, p] = f32(w16)
            LW[c, 8 + b, p] = f32(w16)
            th_h = f16(th)
            th_l = f16(f32(th) - f32(th_h))
            T2[c, p] = f32(np.float64(f32(th_h)) * np.float64(f32(w16))
                           + np.float64(f32(th_l)) * np.float64(f32(w16)))
            LA[c, p, b] = f32(f16(f32(0.4) * (f32(1.0) / f32(k))))
    return LK, LW, T2, LA


def host_constants():
    frac, w0 = _upsample_consts()
    kv = np.arange(1, NH + 1, dtype=np.float32)
    amp = (np.float32(0.4) * (np.float32(1.0) / kv).astype(np.float32)).astype(np.float32)
    A_h, D_h, I_h = _build_filter_mats(256, IR_H, FFT_H, OUT_H)
    A_n, D_n, I_n = _build_filter_mats(80, IR_N, FFT_N, OUT_N)
    LK, LW, T2, LA = _osc_pack()
    return dict(FRAC_full=frac, W0_full=w0, KROW=kv, THRROW=_mask_thresholds(),
                AMPROW=amp, IOTA128=np.arange(128, dtype=np.float32),
                A_h=A_h, D_h=D_h, I_h=I_h, A_n=A_n, D_n=D_n, I_n=I_n,
                LK=LK.reshape(40, 128), LW=LW.reshape(80, 128),
                T2=T2.reshape(5, 128), LA=LA.reshape(640, 8))


# ---------------------------------------------------------------- kernel build
def build(debug=False):
    nc = bacc.Bacc("TRN2", target_bir_lowering=False, debug=False)

    def din(name, shape, dt=F32):
        return nc.dram_tensor(name, list(shape), dt, kind="ExternalInput")

    f0_xp = din("f0_xp", [T + 1])
    f0_win = din("f0_win", [FW + 1])
    mel_win = din("mel_win", [FW, 80])
    phon_win = din("phon_win", [FW])
    sid1 = din("sid1", [1])
    lid1 = din("lid1", [1])
    noise_win = din("noise_win", [FW, HOP])
    framemask = din("framemask", [FW])
    ptab = din("ptab", [128, 128])
    LKd = din("LK", [40, 128])
    LWd = din("LW", [80, 128])
    T2d = din("T2", [5, 128])
    LAd = din("LA", [640, 8])
    WOFCd = din("WOFC", [8], I32)
    sgtab = din("sgtab", [10, 16])
    lgtab = din("lgtab", [5, 8])
    W1 = din("W1", [234, 256])
    b1 = din("b1", [256])
    W2 = din("W2", [256, 336])
    b2 = din("b2", [336])
    FRACf = din("FRAC_full", [T, HOP])
    W0f = din("W0_full", [T, HOP])
    FRACw = din("FRAC_win", [FW, HOP])
    W0w = din("W0_win", [FW, HOP])
    KROW = din("KROW", [NH])
    THRROW = din("THRROW", [NH])
    AMPROW = din("AMPROW", [NH])
    IOTA = din("IOTA128", [128])
    A_h = din("A_h", [256, 2 * NB_H])
    D_h = din("D_h", [HOP, 2 * NB_H])
    I_h = din("I_h", [2 * NB_H, OUT_H])
    A_n = din("A_n", [80, 2 * NB_N])
    D_n = din("D_n", [HOP, 2 * NB_N])
    I_n = din("I_n", [2 * NB_N, OUT_N])
    qb = nc.dram_tensor("qb", [120 * 1024], F32)
    l0d = nc.dram_tensor("l0d", [7680], F32)
    bp1d = nc.dram_tensor("bp1d", [480], F32)
    bp2d = nc.dram_tensor("bp2d", [30], F32)
    vd = nc.dram_tensor("vd", [7680], F32)
    F16 = mybir.dt.float16
    U16 = mybir.dt.uint16
    U32 = mybir.dt.uint32
    PAD = 480                  # prepad samples so window start 240*g0 >= 0
    cfp_d = nc.dram_tensor("cfp_d", [PAD + 120 * 1024], F32)   # packed f16 hi|lo
    puh_d = nc.dram_tensor("puh_d", [FW * HOP], F16)
    pul_d = nc.dram_tensor("pul_d", [FW * HOP], F16)
    hb = nc.dram_tensor("hb", [FW * HOP], F32)
    out_d = nc.dram_tensor("out", [FPC, HOP], F32, kind="ExternalOutput")
    if debug:
        dbg_C = nc.dram_tensor("dbg_C", [120, 1024], F32, kind="ExternalOutput")
        dbg_harm = nc.dram_tensor("dbg_harm", [FW, HOP], F32, kind="ExternalOutput")
        dbg_mag = nc.dram_tensor("dbg_mag", [336, FW], F32, kind="ExternalOutput")
        dbg_cfp = nc.dram_tensor("dbg_cfp", [16, 8, 480], F32, kind="ExternalOutput")
        dbg_pu = nc.dram_tensor("dbg_pu", [16, 16, 480], mybir.dt.float16, kind="ExternalOutput")
        dbg_fr = nc.dram_tensor("dbg_fr", [5, 128, 480], F32, kind="ExternalOutput")
        dbg_sn = nc.dram_tensor("dbg_sn", [5, 128, 480], mybir.dt.float16, kind="ExternalOutput")

    with tile.TileContext(nc) as tc, ExitStack() as ctx:
        cp = ctx.enter_context(tc.tile_pool(name="consts", bufs=1))
        wp = ctx.enter_context(tc.tile_pool(name="work", bufs=1))
        w2p = ctx.enter_context(tc.tile_pool(name="work2", bufs=2))
        op = ctx.enter_context(tc.tile_pool(name="osc", bufs=2))

        # ---------------- consts
        def crow(name, src, n):
            t_ = cp.tile([1, n], F32, tag=name, name=name)
            nc.sync.dma_start(t_[:], src.ap().unsqueeze(0))
            return t_
        def pbc(name, row, p=128):
            t_ = cp.tile([p, row.shape[-1]], F32, tag=name, name=name)
            nc.gpsimd.partition_broadcast(t_[:], row[:] if hasattr(row, 'shape') else row)
            return t_
        iota_row = crow("c_ir", IOTA, 128)
        iota_col = cp.tile([128, 1], F32, tag="c_ic")
        nc.sync.dma_start(iota_col[:], bass.AP(IOTA, 0, [[1, 128], [1, 1]]))
        iota_rows = pbc("c_irs", iota_row)
        ident = cp.tile([128, 128], F32, tag="c_id")
        nc.vector.tensor_scalar(ident[:], iota_rows[:], iota_col[:], None, OP.is_equal)

        # oscillator packing consts (load f32, cast to f16 once)
        LK16, LW16, LA16, T2c = [], [], [], []
        for c in range(5):
            lk32 = cp.tile([8, 128], F32, tag=f"c_lk32_{c}")
            nc.sync.dma_start(lk32[:], LKd.ap()[8 * c:8 * c + 8, :])
            lk16 = cp.tile([8, 128], F16, tag=f"c_lk16_{c}", name=f"c_lk16_{c}")
            nc.vector.tensor_copy(lk16[:], lk32[:])
            LK16.append(lk16)
            lw32 = cp.tile([16, 128], F32, tag=f"c_lw32_{c}")
            nc.sync.dma_start(lw32[:], LWd.ap()[16 * c:16 * c + 16, :])
            lw16 = cp.tile([16, 128], F16, tag=f"c_lw16_{c}", name=f"c_lw16_{c}")
            nc.vector.tensor_copy(lw16[:], lw32[:])
            LW16.append(lw16)
            la32 = cp.tile([128, 8], F32, tag=f"c_la32_{c}")
            nc.sync.dma_start(la32[:], LAd.ap()[128 * c:128 * c + 128, :])
            la16 = cp.tile([128, 8], F16, tag=f"c_la16_{c}", name=f"c_la16_{c}")
            nc.vector.tensor_copy(la16[:], la32[:])
            LA16.append(la16)
            t2 = cp.tile([128, 1], F32, tag=f"c_t2_{c}", name=f"c_t2_{c}")
            nc.sync.dma_start(t2[:], bass.AP(T2d, 128 * c, [[1, 128], [1, 1]]))
            T2c.append(t2)

        def col_chunks(name, src, total):
            outs = []
            base = 0
            i = 0
            while base < total:
                rows = min(128, total - base)
                t_ = cp.tile([128, 1], F32, tag=f"{name}{i}", name=f"{name}{i}")
                if rows < 128:
                    nc.vector.memset(t_[:], 0.0)
                nc.sync.dma_start(t_[:rows], bass.AP(src, base, [[1, rows], [1, 1]]))
                outs.append(t_)
                base += rows
                i += 1
            return outs
        fmcol = col_chunks("c_fm", framemask, FW)
        b1c = col_chunks("c_b1", b1, 256)
        b2c = col_chunks("c_b2", b2, 336)

        def mat_chunks(name, src, rows_total, cols, dt=F32):
            outs = []
            base = 0
            i = 0
            while base < rows_total:
                rows = min(128, rows_total - base)
                if dt == F16:
                    st_ = w2p.tile([128, cols], F32, tag="mstage")
                    nc.sync.dma_start(st_[:rows], src.ap()[base:base + rows, :])
                    t_ = cp.tile([rows, cols], F16, tag=f"{name}{i}", name=f"{name}{i}")
                    nc.scalar.copy(t_[:], st_[:rows])
                else:
                    t_ = cp.tile([rows, cols], dt, tag=f"{name}{i}", name=f"{name}{i}")
                    nc.sync.dma_start(t_[:], src.ap()[base:base + rows, :])
                outs.append(t_)
                base += rows
                i += 1
            return outs
        Ah_t = mat_chunks("c_Ah", A_h, 256, 2 * NB_H, F16)
        Dh_t = mat_chunks("c_Dh", D_h, HOP, 2 * NB_H, F16)
        Ih_t = mat_chunks("c_Ih", I_h, 2 * NB_H, OUT_H, F16)
        An_t = mat_chunks("c_An", A_n, 80, 2 * NB_N, F16)
        Dn_t = mat_chunks("c_Dn", D_n, HOP, 2 * NB_N, F16)
        In_t = mat_chunks("c_In", I_n, 2 * NB_N, OUT_N, F16)
        W1_t = mat_chunks("c_W1", W1, 234, 256, F16)
        W2_t = mat_chunks("c_W2", W2, 256, 336, F16)
        ptab_t = mat_chunks("c_pt", ptab, 128, 128, F16)[0]
        sg_t = mat_chunks("c_sg", sgtab, 10, 16, F16)[0]
        lg_t = mat_chunks("c_lg", lgtab, 5, 8, F16)[0]

        # ---------------- helpers
        def clean_col(tag, src_dram, offset, rows):
            dst = w2p.tile([128, 1], F32, tag=tag, name=tag)
            if rows < 128:
                nc.vector.memset(dst[:], 0.0)
            nc.sync.dma_start(dst[:rows], bass.AP(src_dram, offset, [[1, rows], [1, 1]]))
            nc.vector.tensor_scalar(dst[:], dst[:], 1000.0, None, OP.min)
            nc.vector.tensor_scalar(dst[:], dst[:], 0.0, None, OP.max)
            m = w2p.tile([128, 1], F32, tag="ccm")
            nc.vector.tensor_scalar(m[:], dst[:], 80.0, None, OP.is_ge)
            nc.vector.tensor_tensor(dst[:], dst[:], m[:], OP.mult)
            return dst

        # fence helper: after DMAs that READ `views` (int-bitcast APs), returns
        # an [8,1] I32 zero col available only once those DMAs completed.
        # Mechanism: a write into each DMA's SBUF source is a tracked WAR
        # hazard, so it waits for the DMA; the zero col then reads it (RAW).
        def dma_fence(views, ztag):
            zcol = wp.tile([8, 1], I32, tag=ztag, name=ztag)
            nc.vector.memset(zcol[:], 0)
            for v in views:
                rows = v.shape[0]
                nc.vector.tensor_scalar(v, v, 0, None, OP.bitwise_or)
                zr = w2p.tile([8, 1], I32, tag="fzr")
                if rows < 8:
                    nc.vector.memset(zr[:], 0)
                nc.vector.tensor_scalar(zr[0:rows], v, 0, None, OP.mult)
                nc.vector.tensor_tensor(zcol[:], zcol[:], zr[:], OP.bitwise_or)
            return zcol

        def pitch_up_chunk(src_dram, w0_dram, fr_dram, base, rows, out_tile):
            p0 = clean_col("p0", src_dram, base, rows)
            p1 = clean_col("p1", src_dram, base + 1, rows)
            w0t = w2p.tile([128, HOP], F32, tag="w0t")
            nc.sync.dma_start(w0t[:rows], w0_dram.ap()[base:base + rows, :])
            frt = w2p.tile([128, HOP], F32, tag="frt")
            nc.sync.dma_start(frt[:rows], fr_dram.ap()[base:base + rows, :])
            t0 = w2p.tile([128, HOP], F32, tag="t0")
            nc.vector.tensor_scalar(t0[:rows], w0t[:rows], p0[0:rows, :], None, OP.mult)
            t1 = w2p.tile([128, HOP], F32, tag="t1")
            nc.vector.tensor_scalar(t1[:rows], frt[:rows], p1[0:rows, :], None, OP.mult)
            nc.vector.tensor_tensor(out_tile, t0[:rows], t1[:rows], OP.add)

        # ---------------- S1: full pitch chain -> q -> qb (DRAM)
        for (base, rows) in ((0, 128), (128, 128), (256, 128), (384, 116)):
            pu = w2p.tile([128, HOP], F32, tag="pu")
            pitch_up_chunk(f0_xp, W0f, FRACf, base, rows, pu[:rows])
            qt = w2p.tile([128, HOP], F32, tag="qt")
            # exact fp32 division by SR: q0 = p*r, then Markstein residual correction
            R_SR = float(np.float32(1.0) / np.float32(SR))
            nc.vector.tensor_scalar(qt[:rows], pu[:rows], R_SR, None, OP.mult)
            q0h = w2p.tile([128, HOP], F32, tag="q0h")
            nc.vector.tensor_scalar(q0h[:rows].bitcast(mybir.dt.uint32), qt[:rows].bitcast(mybir.dt.uint32),
                                    0xFFFFF000, None, OP.bitwise_and)
            q0l = w2p.tile([128, HOP], F32, tag="q0l")
            nc.vector.tensor_tensor(q0l[:rows], qt[:rows], q0h[:rows], OP.subtract)
            nc.vector.tensor_scalar(q0h[:rows], q0h[:rows], float(-SR), None, OP.mult)
            nc.vector.tensor_scalar(q0l[:rows], q0l[:rows], float(-SR), None, OP.mult)
            rho = w2p.tile([128, HOP], F32, tag="rho")
            nc.vector.tensor_tensor(rho[:rows], pu[:rows], q0h[:rows], OP.add)
            nc.vector.tensor_tensor(rho[:rows], rho[:rows], q0l[:rows], OP.add)
            nc.vector.tensor_scalar(rho[:rows], rho[:rows], R_SR, None, OP.mult)
            nc.vector.tensor_tensor(qt[:rows], qt[:rows], rho[:rows], OP.add)
            nc.sync.dma_start(bass.AP(qb, base * HOP, [[HOP, rows], [1, HOP]]), qt[:rows])
        zt = wp.tile([120, 24], F32, tag="zt")
        nc.vector.memset(zt[:], 0.0)
        nc.sync.dma_start(bass.AP(qb, 120000, [[24, 120], [1, 24]]), zt[:])

        # ---------------- S2: XLA blocked-16 cumsum on [120, 1024]
        qt2 = wp.tile([120, 1024], F32, tag="csA")
        nc.sync.dma_start(qt2[:], bass.AP(qb, 0, [[1024, 120], [1, 1024]]))
        sm = wp.tile([120, 1024], F32, tag="csB")
        nc.vector.memset(sm[:], 1.0)
        nc.vector.memset(sm[:][:, 0:1024:16], 0.0)
        s0 = wp.tile([120, 1024], F32, tag="csC")
        nc.vector.tensor_tensor_scan(s0[:], sm[:], qt2[:], 0.0, OP.mult, OP.add)
        s0c = wp.tile([120, 64], F32, tag="cs_s0c")
        nc.vector.tensor_copy(s0c[:], s0[:][:, 15:1024:16])
        nc.sync.dma_start(bass.AP(l0d, 0, [[64, 120], [1, 64]]), s0c[:])
        # level 1: scan of L0-block sums (7680 = 60 x 128), XLA-nested
        l0r = wp.tile([60, 128], F32, tag="cs_l0r")
        nc.sync.dma_start(l0r[:], bass.AP(l0d, 0, [[128, 60], [1, 128]]))
        in1 = wp.tile([60, 128], F32, tag="cs_in1")
        nc.vector.tensor_tensor_scan(in1[:], sm[0:60, 0:128], l0r[:], 0.0, OP.mult, OP.add)
        # level 2: scan of L1-block sums (480 = 30 x 16)
        in1c = wp.tile([60, 8], F32, tag="cs_in1c")
        nc.vector.tensor_copy(in1c[:], in1[:][:, 15:128:16])
        l1r = wp.tile([30, 16], F32, tag="cs_l1r")
        nc.sync.dma_start(l1r[:], in1c[:])
        in2 = wp.tile([30, 16], F32, tag="cs_in2")
        nc.vector.tensor_tensor_scan(in2[:], sm[0:30, 0:16], l1r[:], 0.0, OP.mult, OP.add)
        # level 3: scan of L2-block sums (30)
        l2r = wp.tile([1, 30], F32, tag="cs_l2r")
        nc.sync.dma_start(l2r[:], in2[:][:, 15:16])
        in3 = wp.tile([1, 30], F32, tag="cs_in3")
        nc.vector.tensor_tensor_scan(in3[:], sm[0:1, 0:30], l2r[:], 0.0, OP.mult, OP.add)
        # bpref2 (inclusive scanned L2-sums, L3/L4-nested): x4p + in3
        x4p = wp.tile([1, 30], F32, tag="cs_x4")
        nc.vector.memset(x4p[:], 0.0)
        nc.vector.tensor_copy(x4p[:][:, 16:30], in3[:][:, 15:16].broadcast_to((1, 14)))
        bp2 = wp.tile([1, 30], F32, tag="cs_bp2")
        nc.vector.tensor_tensor(bp2[:], x4p[:], in3[:], OP.add)
        nc.sync.dma_start(bass.AP(bp2d, 0, [[30, 1], [1, 30]]), bp2[:])
        # bpref1 [30,16] = fl(bp2shift_col + in2)
        bp2s = wp.tile([30, 1], F32, tag="cs_bp2s")
        nc.vector.memset(bp2s[:], 0.0)
        nc.sync.dma_start(bp2s[1:30, :], bass.AP(bp2d, 0, [[1, 29], [1, 1]]))
        bp1 = wp.tile([30, 16], F32, tag="cs_bp1")
        nc.vector.tensor_scalar(bp1[:], in2[:], bp2s[:], None, OP.add)
        nc.sync.dma_start(bass.AP(bp1d, 0, [[16, 30], [1, 16]]), bp1[:])
        # bpref0 [60,128] = fl(bp1shift_grp + in1)
        bp1s = wp.tile([60, 8], F32, tag="cs_bp1s")
        nc.vector.memset(bp1s[:], 0.0)
        nc.sync.dma_start(bp1s[:, 1:8], bass.AP(bp1d, 0, [[8, 60], [1, 7]]))
        nc.sync.dma_start(bp1s[1:60, 0:1], bass.AP(bp1d, 7, [[8, 59], [1, 1]]))
        bp0 = wp.tile([60, 128], F32, tag="cs_bp0")
        nc.vector.tensor_tensor(bp0[:].rearrange("p (g j) -> p g j", j=16),
                                in1[:].rearrange("p (g j) -> p g j", j=16),
                                bp1s[:].unsqueeze(2).broadcast_to((60, 8, 16)), OP.add)
        # V[b0] = bp0[b0-1] (global shift by one block)
        vt = wp.tile([60, 128], F32, tag="cs_vt")
        nc.vector.memset(vt[:][:, 0:1], 0.0)
        nc.vector.tensor_copy(vt[:][:, 1:128], bp0[:][:, 0:127])
        nc.sync.dma_start(vt[1:60, 0:1], bp0[0:59, 127:128])
        nc.sync.dma_start(bass.AP(vd, 0, [[128, 60], [1, 128]]), vt[:])
        vcol = wp.tile([120, 64], F32, tag="cs_vcol")
        nc.sync.dma_start(vcol[:], bass.AP(vd, 0, [[64, 120], [1, 64]]))
        Ct = wp.tile([120, 1024], F32, tag="csD")
        nc.vector.tensor_tensor(Ct[:].rearrange("p (g j) -> p g j", j=16),
                                s0[:].rearrange("p (g j) -> p g j", j=16),
                                vcol[:].unsqueeze(2).broadcast_to((120, 64, 16)), OP.add)
        if debug:
            nc.sync.dma_start(dbg_C.ap(), Ct[:])

        # ---------------- S3: dd + Cf2 -> cfb
        phi = wp.tile([120, 1024], F32, tag="csA")
        nc.vector.tensor_scalar(phi[:], Ct[:], float(H_F), None, OP.mult)
        ch = wp.tile([120, 1024], F32, tag="csB")
        nc.vector.tensor_scalar(ch[:].bitcast(mybir.dt.uint32), Ct[:].bitcast(mybir.dt.uint32),
                                0xFFFFF000, None, OP.bitwise_and)
        cl = wp.tile([120, 1024], F32, tag="csC")
        nc.vector.tensor_tensor(cl[:], Ct[:], ch[:], OP.subtract)
        e = wp.tile([120, 1024], F32, tag="csE")
        nc.vector.tensor_scalar(e[:], ch[:], float(HH_F), None, OP.mult)
        nc.vector.tensor_tensor(e[:], e[:], phi[:], OP.subtract)
        tmp = wp.tile([120, 1024], F32, tag="csF")
        nc.vector.tensor_scalar(tmp[:], cl[:], float(HH_F), None, OP.mult)
        nc.vector.tensor_tensor(e[:], e[:], tmp[:], OP.add)
        nc.vector.tensor_scalar(tmp[:], ch[:], float(HL_F), None, OP.mult)
        nc.vector.tensor_tensor(e[:], e[:], tmp[:], OP.add)
        nc.vector.tensor_scalar(tmp[:], cl[:], float(HL_F), None, OP.mult)
        nc.vector.tensor_tensor(e[:], e[:], tmp[:], OP.add)
        nc.vector.tensor_scalar(tmp[:], Ct[:], float(EPSH_F), None, OP.mult)
        nc.vector.tensor_tensor(tmp[:], tmp[:], e[:], OP.subtract)
        nc.vector.tensor_scalar(tmp[:], tmp[:], float(INV2PI_F), None, OP.mult)
        cfr = wp.tile([120, 1024], F32, tag="csD2")
        fl_ = wp.tile([120, 1024], F32, tag="csFL")
        nc.vector.tensor_scalar(fl_[:], Ct[:], float(2.0 ** 23), None, OP.add)
        nc.vector.tensor_scalar(fl_[:], fl_[:], float(2.0 ** 23), None, OP.subtract)
        gg = wp.tile([120, 1024], F32, tag="csGG")
        nc.vector.tensor_tensor(gg[:], fl_[:], Ct[:], OP.is_gt)
        nc.vector.tensor_tensor(fl_[:], fl_[:], gg[:], OP.subtract)
        nc.vector.tensor_tensor(cfr[:], Ct[:], fl_[:], OP.subtract)
        nc.vector.tensor_tensor(cfr[:], cfr[:], tmp[:], OP.add)
        # cf -> f16 hi/lo split packed into one f32 word per sample -> DRAM
        cfh16 = wp.tile([120, 1024], F16, tag="cfh16")
        nc.vector.tensor_copy(cfh16[:], cfr[:])
        cfhf = wp.tile([120, 1024], F32, tag="csE")
        nc.scalar.copy(cfhf[:], cfh16[:])
        cflf = wp.tile([120, 1024], F32, tag="csF")
        nc.vector.tensor_tensor(cflf[:], cfr[:], cfhf[:], OP.subtract)
        cfl16 = wp.tile([120, 1024], F16, tag="cfl16")
        nc.vector.tensor_copy(cfl16[:], cflf[:])
        cfph = wp.tile([120, 1024], U32, tag="csE")
        nc.vector.tensor_copy(cfph[:], cfh16[:].bitcast(U16))
        cfpl = wp.tile([120, 1024], U32, tag="csF")
        nc.vector.tensor_copy(cfpl[:], cfl16[:].bitcast(U16))
        nc.vector.tensor_scalar(cfpl[:], cfpl[:], 16, None, OP.logical_shift_left)
        nc.vector.tensor_tensor(cfph[:], cfph[:], cfpl[:], OP.bitwise_or)
        zpad = wp.tile([1, PAD], F32, tag="zpad")
        nc.vector.memset(zpad[:], 0.0)
        nc.sync.dma_start(bass.AP(cfp_d, 0, [[PAD, 1], [1, PAD]]), zpad[:])
        nc.sync.dma_start(bass.AP(cfp_d, PAD, [[1024, 120], [1, 1024]]),
                          cfph[:].bitcast(F32))
        z0 = dma_fence([cfph[:].bitcast(I32)[0:8, 0:1],
                        zpad[:].bitcast(I32)[0:1, 0:1]], "z0cf")

        # ---------------- S4: window pitch f16 hi/lo -> DRAM rows
        pu_srcs = []
        for fc in range(2):
            base = fc * 128
            puw = wp.tile([128, HOP], F32, tag=f"puw{fc}")
            pitch_up_chunk(f0_win, W0w, FRACw, base, 128, puw[:])
            puh16 = w2p.tile([128, HOP], F16, tag="puh16")
            nc.vector.tensor_copy(puh16[:], puw[:])
            puhf = w2p.tile([128, HOP], F32, tag="puhf")
            nc.scalar.copy(puhf[:], puh16[:])
            pulf = w2p.tile([128, HOP], F32, tag="pulf")
            nc.vector.tensor_tensor(pulf[:], puw[:], puhf[:], OP.subtract)
            pul16 = w2p.tile([128, HOP], F16, tag="pul16")
            nc.vector.tensor_copy(pul16[:], pulf[:])
            nc.sync.dma_start(bass.AP(puh_d, base * HOP, [[HOP, 128], [1, HOP]]), puh16[:])
            nc.sync.dma_start(bass.AP(pul_d, base * HOP, [[HOP, 128], [1, HOP]]), pul16[:])
            pu_srcs.append(puh16[:].bitcast(I32)[0:8, 0:1])
            pu_srcs.append(pul16[:].bitcast(I32)[0:8, 0:1])
        z0pu = dma_fence(pu_srcs, "z0pu")

        # ---------------- S4b: PE-centric oscillator sweep
        # rhs rows: blocks b=0..7 are 32-frame spans; row b covers window
        # samples [7680b, 7680b+7680). Window sample w corresponds to full-row
        # sample 240*g0 + w -> cfh_d offset PAD + 240*g0 + w.
        BL = 7680                  # samples per block
        L = 480                    # tile length (indirect offsets are L-units)
        NT = BL // L               # 16 tiles
        # per-core window start: host passes WOFC[b] = 125h + 16b (480-sample
        # units into the padded buffer); tile t adds +t
        wofc = wp.tile([8, 1], I32, tag="wofc")
        nc.sync.dma_start(wofc[:], bass.AP(WOFCd, 0, [[1, 8], [1, 1]]))
        oscstack = ExitStack()
        orp = oscstack.enter_context(tc.tile_pool(name="oscrows", bufs=2))
        ohp = oscstack.enter_context(tc.tile_pool(name="oscout", bufs=2))
        opsW = oscstack.enter_context(tc.tile_pool(name="opsW", bufs=2, space="PSUM"))
        opsP = oscstack.enter_context(tc.tile_pool(name="opsP", bufs=2, space="PSUM"))
        opsO = oscstack.enter_context(tc.tile_pool(name="opsO", bufs=2, space="PSUM"))
        hr_refs = []
        for t in range(NT):
            cfp_t = orp.tile([8, L], F32, tag="o_cfp")
            wofct = orp.tile([8, 1], I32, tag="o_wofct")
            nc.vector.tensor_scalar(wofct[:], wofc[:], t, None, OP.add)
            nc.vector.tensor_tensor(wofct[:], wofct[:], z0[:], OP.add)
            nc.gpsimd.indirect_dma_start(
                cfp_t[:], None, bass.AP(cfp_d, 0, [[L, 256], [1, L]]),
                IndirectOffsetOnAxis(ap=wofct[:], axis=0))
            cf16v = cfp_t[:].bitcast(F16)
            pu_t = orp.tile([16, L], F16, tag="o_pu")
            nc.vector.tensor_copy(pu_t[:].bitcast(I32)[0:8, 0:1], z0pu[:])
            nc.sync.dma_start(pu_t[0:8, :], bass.AP(puh_d, L * t, [[BL, 8], [1, L]]))
            nc.sync.dma_start(pu_t[8:16, :], bass.AP(pul_d, L * t, [[BL, 8], [1, L]]))
            if debug:
                nc.sync.dma_start(bass.AP(dbg_cfp, t * 8 * L, [[L, 8], [1, L]]),
                                  cfp_t[:])
                nc.sync.dma_start(bass.AP(dbg_pu, t * 16 * L, [[L, 16], [1, L]]),
                                  pu_t[:])
            psO = opsO.tile([8, L], F32, tag="psO")
            for c in range(5):
                psW = opsW.tile([128, L], F32, tag="psW")
                nc.tensor.matmul(psW[:], LK16[c][:], cf16v[:, 0:2 * L:2], start=True, stop=False)
                nc.tensor.matmul(psW[:], LK16[c][:], cf16v[:, 1:2 * L:2], start=False, stop=True)
                psP = opsP.tile([128, L], F32, tag="psP")
                nc.tensor.matmul(psP[:], LW16[c][:], pu_t[:], start=True, stop=True)
                mt = op.tile([128, L], F32, tag="o_mt")
                nc.vector.tensor_scalar(mt[:], psP[:], T2c[c][:], None, OP.is_lt)
                wtm = op.tile([128, L], F32, tag="o_wtm")
                nc.vector.tensor_tensor(wtm[:], psW[:], mt[:], OP.mult)
                rnd = op.tile([128, L], F32, tag="o_rnd")
                nc.gpsimd.tensor_scalar(rnd[:], wtm[:], float(2.0 ** 23), float(2.0 ** 23),
                                        OP.add, OP.subtract)
                fr = op.tile([128, L], F32, tag="o_mt")
                nc.vector.tensor_tensor(fr[:], wtm[:], rnd[:], OP.subtract)
                sn = op.tile([128, L], F16, tag="o_sn")
                nc.scalar.activation(sn[:], fr[:], AF.Sin, scale=TWO_PI_F)
                nc.tensor.matmul(psO[:], LA16[c][:], sn[:], start=(c == 0), stop=(c == 4))
                if debug and t == 0:
                    nc.sync.dma_start(
                        bass.AP(dbg_fr, c * 128 * L, [[L, 128], [1, L]]), fr[:])
                    nc.sync.dma_start(
                        bass.AP(dbg_sn, c * 128 * L, [[L, 128], [1, L]]), sn[:])
            hr_t = ohp.tile([8, L], F32, tag="o_hr")
            nc.scalar.copy(hr_t[:], psO[:])
            nc.sync.dma_start(bass.AP(hb, L * t, [[BL, 8], [1, L]]), hr_t[:])
            hr_refs.append(hr_t)
        z0hb = dma_fence([hr_refs[-1][:].bitcast(I32)[0:8, 0:1],
                          hr_refs[-2][:].bitcast(I32)[0:8, 0:1]], "z0hb")

        oscstack.close()
        pp = ctx.enter_context(tc.tile_pool(name="psum", bufs=4, space="PSUM"))
        py = ctx.enter_context(tc.tile_pool(name="psumy", bufs=1, space="PSUM"))

        # ---------------- back to frame-major [128, 240] chunks
        M1 = [wp.tile([128, HOP], F32, tag=f"m1_{fc}", name=f"m1_{fc}") for fc in range(2)]
        for fc in range(2):
            nc.vector.tensor_copy(M1[fc][:].bitcast(I32)[0:8, 0:1], z0hb[:])
            nc.sync.dma_start(M1[fc][:], bass.AP(hb, fc * 128 * HOP, [[HOP, 128], [1, HOP]]))
        if debug:
            nc.sync.dma_start(dbg_harm.ap()[0:128, :], M1[0][:])
            nc.sync.dma_start(dbg_harm.ap()[128:256, :], M1[1][:])

        # ---------------- S5: framesT via PE transpose (harm + noise) -> f16
        def transpose_fw(src_tiles, name):
            d0 = wp.tile([128, FW], F16, tag=f"{name}0")
            d1 = wp.tile([112, FW], F16, tag=f"{name}1")
            for fc in range(2):
                ps = pp.tile([128, 128], F32, tag="ps")
                nc.tensor.transpose(ps[:], src_tiles[fc][:][:, 0:128], ident[:])
                nc.scalar.copy(d0[:][:, fc * 128:(fc + 1) * 128], ps[:])
                ps2 = pp.tile([112, 128], F32, tag="ps")
                nc.tensor.transpose(ps2[:], src_tiles[fc][:][:, 128:240], ident[:])
                nc.scalar.copy(d1[:][:, fc * 128:(fc + 1) * 128], ps2[:])
            return d0, d1

        HFT0, HFT1 = transpose_fw(M1, "hft")
        NZM = [wp.tile([128, HOP], F32, tag=f"nzm{fc}", name=f"nzm{fc}") for fc in range(2)]
        for fc in range(2):
            nz = w2p.tile([128, HOP], F32, tag="nzin")
            nc.sync.dma_start(nz[:], noise_win.ap()[fc * 128:(fc + 1) * 128, :])
            nc.vector.tensor_scalar(NZM[fc][:], nz[:], 2.0, 1.0, OP.mult, OP.subtract)
        NFT0, NFT1 = transpose_fw(NZM, "nft")

        # ---------------- S6: MLP -> magT (f16 matmuls)
        melT = wp.tile([80, FW], F16, tag="melT")
        for fc in range(2):
            melc = w2p.tile([128, 80], F32, tag="melc")
            nc.sync.dma_start(melc[:], mel_win.ap()[fc * 128:(fc + 1) * 128, :])
            ps = pp.tile([80, 128], F32, tag="ps")
            nc.tensor.transpose(ps[:], melc[:], ident[:])
            nc.scalar.copy(melT[:][:, fc * 128:(fc + 1) * 128], ps[:])
        f0row = wp.tile([1, FW], F32, tag="f0row")
        nc.sync.dma_start(f0row[:], bass.AP(f0_win, 0, [[FW, 1], [1, FW]]))
        f0h16 = wp.tile([1, FW], F16, tag="f0h16")
        nc.vector.tensor_copy(f0h16[:], f0row[:])
        f0hf = wp.tile([1, FW], F32, tag="f0hf")
        nc.vector.tensor_copy(f0hf[:], f0h16[:])
        f0lf = wp.tile([1, FW], F32, tag="f0lf")
        nc.vector.tensor_tensor(f0lf[:], f0row[:], f0hf[:], OP.subtract)
        f0l16 = wp.tile([1, FW], F16, tag="f0l16")
        nc.vector.tensor_copy(f0l16[:], f0lf[:])
        phrow = wp.tile([1, FW], F32, tag="phrow")
        nc.sync.dma_start(phrow[:], bass.AP(phon_win, 0, [[FW, 1], [1, FW]]))
        phrows = wp.tile([128, FW], F32, tag="phrows")
        nc.gpsimd.partition_broadcast(phrows[:], phrow[:])
        onehot = wp.tile([128, FW], F16, tag="onehot")
        nc.vector.tensor_scalar(onehot[:], phrows[:], iota_col[:], None, OP.is_equal)
        phps = pp.tile([128, FW], F32, tag="ps")
        nc.tensor.matmul(phps[:], ptab_t[:], onehot[:], start=True, stop=True)
        phT = wp.tile([128, FW], F16, tag="phT")
        nc.scalar.copy(phT[:], phps[:])

        def emb_bcast(tab_tile, idx_dram, nrows, dim, name):
            idxb = wp.tile([nrows, 1], F32, tag=f"{name}i")
            nc.sync.dma_start(idxb[:], bass.AP(idx_dram, 0, [[0, nrows], [1, 1]]))
            oh = wp.tile([nrows, 1], F16, tag=f"{name}o")
            nc.vector.tensor_scalar(oh[:], iota_col[0:nrows, :], idxb[:], None, OP.is_equal)
            vps = pp.tile([dim, 1], F32, tag="ps")
            nc.tensor.matmul(vps[:], tab_tile[:], oh[:], start=True, stop=True)
            vcol = wp.tile([dim, 1], F16, tag=f"{name}c")
            nc.scalar.copy(vcol[:], vps[:])
            vT = wp.tile([dim, FW], F16, tag=f"{name}T")
            nc.vector.tensor_copy(vT[:], vcol[:].broadcast_to((dim, FW)))
            return vT
        sgT = emb_bcast(sg_t, sid1, 10, 16, "sg")
        lgT = emb_bcast(lg_t, lid1, 5, 8, "lg")

        # featsT f16: chunk0 = [mel(80) | f0h(1) | ph 0:47],
        #             chunk1 = [ph 47:128 | sg | lg | f0l]  (W1 row 80 duplicated at 233)
        ft0 = wp.tile([128, FW], F16, tag="ft0")
        ft1 = wp.tile([106, FW], F16, tag="ft1")
        nc.sync.dma_start(ft0[0:80, :], melT[:])
        nc.sync.dma_start(ft0[80:81, :], f0h16[:])
        nc.sync.dma_start(ft0[81:128, :], phT[0:47, :])
        nc.sync.dma_start(ft1[0:81, :], phT[47:128, :])
        nc.sync.dma_start(ft1[81:97, :], sgT[:])
        nc.sync.dma_start(ft1[97:105, :], lgT[:])
        nc.sync.dma_start(ft1[105:106, :], f0l16[:])
        HT = [wp.tile([128, FW], F16, tag=f"HT{mc}", name=f"HT{mc}") for mc in range(2)]
        for mc in range(2):
            msl = slice(mc * 128, (mc + 1) * 128)
            hps = pp.tile([128, FW], F32, tag="ps")
            nc.tensor.matmul(hps[:], W1_t[0][:, msl], ft0[:], start=True, stop=False)
            nc.tensor.matmul(hps[:], W1_t[1][0:106, msl], ft1[:], start=False, stop=True)
            nc.scalar.activation(HT[mc][:], hps[:], AF.Relu, bias=b1c[mc][:], scale=1.0)
        magT = [wp.tile([128, FW], F32, tag=f"magT{mc}", name=f"magT{mc}") for mc in range(3)]
        magT16 = [wp.tile([128, FW], F16, tag=f"magS{mc}", name=f"magS{mc}") for mc in range(3)]
        for mc, rows in enumerate((128, 128, 80)):
            msl = slice(mc * 128, mc * 128 + rows)
            cps = pp.tile([rows, FW], F32, tag="ps")
            nc.tensor.matmul(cps[:], W2_t[0][:, msl], HT[0][:], start=True, stop=False)
            nc.tensor.matmul(cps[:], W2_t[1][:, msl], HT[1][:], start=False, stop=True)
            mg = magT[mc][0:rows, :]
            nc.scalar.activation(mg, cps[:], AF.Sigmoid, bias=b2c[mc][0:rows, :], scale=1.0)
            nc.scalar.activation(mg, mg, AF.Ln)
            nc.scalar.activation(mg, mg, AF.Exp, scale=LN10_F)
            nc.scalar.activation(magT16[mc][0:rows, :], mg, AF.Copy, bias=1e-7, scale=2.0)
        if debug:
            for mc, rows in enumerate((128, 128, 80)):
                nc.sync.dma_start(dbg_mag.ap()[mc * 128:mc * 128 + rows, :], magT[mc][0:rows, :])

        # ---------------- S7: filters
        def spectrum(lhs, lhs_rows, rhs, nchunks, name):
            outs = []
            for mc in range(nchunks):
                msl = slice(mc * 128, (mc + 1) * 128)
                ps = pp.tile([128, FW], F32, tag="ps")
                for k in range(len(lhs)):
                    nc.tensor.matmul(ps[:], lhs[k][0:lhs_rows[k], msl], rhs[k],
                                     start=(k == 0), stop=(k == len(lhs) - 1))
                o = wp.tile([128, FW], F32, tag=f"{name}{mc}", name=f"{name}{mc}")
                nc.scalar.copy(o[:], ps[:])
                outs.append(o)
            return outs

        SIR_h = spectrum(Ah_t, [128, 128], [magT16[0][:], magT16[1][:]], 6, "sirh")
        SFR_h = spectrum(Dh_t, [128, 112], [HFT0[:], HFT1[:]], 6, "sfrh")
        SIR_n = spectrum(An_t, [80], [magT16[2][0:80, :]], 4, "sirn")
        SFR_n = spectrum(Dn_t, [128, 112], [NFT0[:], NFT1[:]], 4, "sfrn")

        def cmul(a, b, nre, name):
            outs = []
            for c in range(nre * 2):
                outs.append(wp.tile([128, FW], F16, tag=f"{name}{c}", name=f"{name}{c}"))
            for c in range(nre):
                t1_ = w2p.tile([128, FW], F32, tag=f"{name}t1")
                t2_ = w2p.tile([128, FW], F32, tag=f"{name}t2")
                nc.vector.tensor_tensor(t1_[:], a[c][:], b[c][:], OP.mult)
                nc.vector.tensor_tensor(t2_[:], a[c + nre][:], b[c + nre][:], OP.mult)
                nc.vector.tensor_tensor(outs[c][:], t1_[:], t2_[:], OP.subtract)
                t3_ = w2p.tile([128, FW], F32, tag=f"{name}t1")
                t4_ = w2p.tile([128, FW], F32, tag=f"{name}t2")
                nc.vector.tensor_tensor(t3_[:], a[c][:], b[c + nre][:], OP.mult)
                nc.vector.tensor_tensor(t4_[:], a[c + nre][:], b[c][:], OP.mult)
                nc.vector.tensor_tensor(outs[c + nre][:], t3_[:], t4_[:], OP.add)
            return outs

        SY_h = cmul(SIR_h, SFR_h, 3, "cmh")
        SY_n = cmul(SIR_n, SFR_n, 2, "cmn")

        def irfft_y(SY, I_tiles, out_len, name):
            Ysb = []
            for fc in range(2):
                fsl = slice(fc * 128, (fc + 1) * 128)
                yp = py.tile([128, out_len], F32, tag=f"yp_{name}")
                for ns in range(0, out_len, 512):
                    ne = min(out_len, ns + 512)
                    for k in range(len(SY)):
                        nc.tensor.matmul(yp[:][:, ns:ne], SY[k][:][:, fsl], I_tiles[k][:][:, ns:ne],
                                         start=(k == 0), stop=(k == len(SY) - 1))
                o = wp.tile([128, out_len], F32, tag=f"{name}sb{fc}", name=f"{name}sb{fc}")
                nc.vector.tensor_scalar(o[:], yp[:], fmcol[fc][:], None, OP.mult)
                Ysb.append(o)
            return Ysb

        Yh = irfft_y(SY_h, Ih_t, OUT_H, "yh")
        Yn = irfft_y(SY_n, In_t, OUT_N, "yn")

        # ---------------- S8: OLA + output
        for oc_i, orows in ((0, 128), (1, 122)):
            F0 = oc_i * 128
            acc = wp.tile([128, HOP], F32, tag=f"acc{oc_i}", name=f"acc{oc_i}")
            nc.vector.memset(acc[:], 0.0)

            def add_contrib(Y, j, d, out_len):
                pos0 = HOP * j + d
                r0, r1 = max(0, -pos0), min(HOP, out_len - pos0)
                if r0 >= r1:
                    return
                g0_ = F0 + 2 - j
                sh = w2p.tile([128, HOP], F32, tag="olash", name="olash")
                nc.vector.memset(sh[:], 0.0)
                for part in range(2):
                    lo = max(g0_, part * 128) - g0_
                    hi = min(g0_ + orows, (part + 1) * 128) - g0_
                    if lo >= hi:
                        continue
                    nc.sync.dma_start(
                        sh[lo:hi, r0:r1],
                        Y[part][:][g0_ + lo - part * 128: g0_ + hi - part * 128,
                                   pos0 + r0: pos0 + r1])
                nc.vector.tensor_tensor(acc[:], acc[:], sh[:], OP.add)

            for j in (-2, -1, 0, 1, 2):
                add_contrib(Yh, j, IR_H // 2, OUT_H)
            for j in (-1, 0, 1):
                add_contrib(Yn, j, IR_N // 2, OUT_N)
            nc.sync.dma_start(out_d.ap()[F0:F0 + orows, :], acc[0:orows, :])

    nc.compile()
    return nc


# ---------------------------------------------------------------- host driver
_CACHE = {}


def _get_nc(debug=False):
    key = ("nc", debug)
    if key not in _CACHE:
        _CACHE[key] = build(debug=debug)
    return _CACHE[key]


def make_in_maps(inputs, consts=None):
    consts = consts or host_constants()
    f32 = np.float32
    mel = np.asarray(inputs["mel"]).astype(f32)
    f0 = np.asarray(inputs["f0"]).astype(f32)
    phon = np.asarray(inputs["phoneme_seq"]).astype(f32)
    noise = np.asarray(inputs["noise"]).astype(f32)
    ptab = np.zeros((128, 128), f32)
    ptab[:101] = np.asarray(inputs["phoneme_table"]).astype(f32)
    in_maps = []
    for c in range(8):
        b, h = c // 2, c % 2
        g0 = h * FPC - 2
        gidx = np.arange(FW) + g0
        valid = (gidx >= 0) & (gidx < T)
        gcl = np.clip(gidx, 0, T - 1)
        xp = np.concatenate([f0[b], f0[b, -1:]])
        f0w = np.zeros(FW + 1, f32)
        gi2 = np.arange(FW + 1) + g0
        v2 = (gi2 >= 0) & (gi2 < T + 1)
        f0w[v2] = xp[np.clip(gi2, 0, T)][v2]
        melw = np.zeros((FW, 80), f32); melw[valid] = mel[b][gcl[valid]]
        phw = np.zeros(FW, f32); phw[valid] = phon[b][gcl[valid]]
        nzw = np.zeros((FW, HOP), f32)
        nzw[valid] = noise[b].reshape(T, HOP)[gcl[valid]]
        fm = valid.astype(f32)
        m = dict(
            f0_xp=xp.astype(f32), f0_win=f0w, mel_win=melw, phon_win=phw,
            sid1=np.asarray(inputs["singer_id"]).astype(f32)[b:b + 1].copy(),
            lid1=np.asarray(inputs["language_id"]).astype(f32)[b:b + 1].copy(),
            noise_win=nzw, framemask=fm,
            ptab=ptab,
            LK=consts["LK"], LW=consts["LW"], T2=consts["T2"], LA=consts["LA"],
            WOFC=(125 * h + 16 * np.arange(8)).astype(np.int32),
            sgtab=np.asarray(inputs["singer_table"]).astype(f32),
            lgtab=np.asarray(inputs["language_table"]).astype(f32),
            W1=np.vstack([np.asarray(inputs["W1"]).astype(f32),
                          np.asarray(inputs["W1"]).astype(f32)[80:81]]),
            b1=np.asarray(inputs["b1"]).astype(f32),
            W2=np.asarray(inputs["W2"]).astype(f32), b2=np.asarray(inputs["b2"]).astype(f32),
            FRAC_full=consts["FRAC_full"], W0_full=consts["W0_full"],
            FRAC_win=(consts["FRAC_full"][gcl] * fm[:, None]).astype(f32),
            W0_win=(consts["W0_full"][gcl] * fm[:, None]).astype(f32),
            KROW=consts["KROW"], THRROW=consts["THRROW"], AMPROW=consts["AMPROW"],
            IOTA128=consts["IOTA128"],
            A_h=consts["A_h"], D_h=consts["D_h"], I_h=consts["I_h"],
            A_n=consts["A_n"], D_n=consts["D_n"], I_n=consts["I_n"],
        )
        in_maps.append(m)
    return in_maps


def kernel(**inputs):
    nc = _get_nc(debug=False)
    in_maps = make_in_maps(inputs)
    res = run_bass_kernel_spmd(nc, in_maps, list(range(8)))
    out = np.zeros((B, N), np.float32)
    for c in range(8):
        b, h = c // 2, c % 2
        out[b, h * HALF:(h + 1) * HALF] = res.results[c]["out"].reshape(HALF)
    return out

